# revision 1
# baseline (speedup 1.0000x reference)
"""8-core Trainium2 Bass kernel for nn_BolmoLocalLayer.

Strategy (uniform SPMD program, rank-dependence only in data):
 - host: fold norm1 into Wcat/wv, mh_w into w_out, norm2 into w_gate/w_up;
   pre-transpose x per core; pre-cast weights to bf16.
 - token-parallel projections (each core: its 512 tokens, all heads)
 - AllToAll #1: core c receives head c's qT/kT/ogT/v (+ gates) for all tokens
 - head-sharded mLSTM attention (m=0 rescale; n=max(|sum C|,1); eps*n^2
   folded into the per-head RMS norm)
 - w_out partials + ReduceScatter (bf16) -> own-token x1 = x + mix
 - norm2 -> AllGather(h2) -> FF-sharded SwiGLU MLP -> bf16 partial outputs
 - host: scatter x1 rows + sum MLP partials
"""
import sys

for _p in ("/opt/trn_rl_repo", "/root/.axon_site/_ro/trn_rl_repo"):
    if _p not in sys.path:
        sys.path.append(_p)

import numpy as np
import ml_dtypes

import concourse.mybir as mybir
from concourse import bacc
from concourse.tile import TileContext
from concourse.bass_utils import run_bass_kernel_spmd
from concourse.bass import ds

bf16 = ml_dtypes.bfloat16
FP32 = mybir.dt.float32
BF16 = mybir.dt.bfloat16

B, S, D, H = 2, 2048, 2048, 8
QK, FF = D // 2, 8192
dqk, dv = QK // H, D // H        # 128, 256
R = 8                            # cores
OB = S // R                      # 256 own tokens per batch
OT = 2 * OB                      # 512 own tokens
NK = D // 128                    # 16 contraction tiles over D
CAP, EPS = 15.0, 1e-6
FFC = FF // R                    # 1024 ff slice per core
WCOLS = 2 * QK + D + 2 * H       # 4112
# a2a payloads (bf16 elems per dest)
# buf1: qT 128x512 | kT 128x512    bufV: v 4x128x256    bufOG: ogT 2x128x512
PAY_Q, PAY_K, PAY1 = 0, 65536, 131072
PAYV, PAYOG = 131072, 131072

AL = mybir.AluOpType
AF = mybir.ActivationFunctionType


def _mixer(nc, tc, xT, wcat_d, wv_d, b16, a2a1_in, a2a1_out, a2av_in,
           a2av_out, a2aog_in, a2aog_out, ag_g_in, ag_g_out, rg, onesb, ones1f):
    """norm1 + projections for own 512 tokens; write a2a payloads.

    Order: gates + q + k projections first, then fire the gates A2A and the
    q/k A2A so attention's prelude overlaps the og/v projections + their A2A.
    """
    with tc.tile_pool(name="mx_w", bufs=3) as wp, \
         tc.tile_pool(name="mx_wv", bufs=2) as wvp, \
         tc.tile_pool(name="mx_tmp", bufs=3) as tp, \
         tc.tile_pool(name="mx_out", bufs=1) as op, \
         tc.tile_pool(name="mx_ps", bufs=2, space="PSUM") as ps, \
         tc.tile_pool(name="mx_ps1", bufs=1, space="PSUM") as ps1:
        xhat = op.tile([128, NK, OT], BF16)
        # ssq over D (partition axis) via bf16 squares + ones-matmul
        ssq_ps = ps1.tile([1, OT], FP32, tag="ssq1")
        for kt in range(NK):
            sq = tp.tile([128, OT], BF16, tag="sq")
            nc.vector.tensor_tensor(sq[:], xT[:, kt], xT[:, kt], AL.mult)
            nc.tensor.matmul(ssq_ps[:], onesb[:], sq[:],
                             start=(kt == 0), stop=(kt == NK - 1))
        scl = tp.tile([1, OT], FP32, tag="scl1")
        nc.vector.tensor_scalar(scl[:], ssq_ps[:], 1.0 / D, EPS, AL.mult, AL.add)
        nc.scalar.activation(scl[:], scl[:], AF.Ln)
        nc.scalar.activation(scl[:], scl[:], AF.Exp, scale=-0.5)
        sc_ps = ps1.tile([128, OT], FP32, tag="sc1")
        nc.tensor.matmul(sc_ps[:], ones1f[0:1, :], scl[:], start=True, stop=True)
        for kt in range(NK):
            nc.vector.tensor_tensor(xhat[:, kt], xT[:, kt], sc_ps[:], AL.mult)

        # --- Wcat projections: out[F, own-t].  M-tiles: 0..7 q | 8..15 k |
        # 16..31 og | 32 gates(16 cols, [i0 f0 i1 f1 ...])
        qT = op.tile([128, H, OT], BF16)
        kT = op.tile([128, H, OT], BF16)
        ogT = op.tile([128, NK, OT], BF16)
        wc3 = wcat_d.rearrange("(kt p) f -> p kt f", p=128)
        for m in [32] + list(range(16)) + ["v"] + list(range(16, 32)):
            if m == "v":
                _v_proj(nc, xhat, wv_d, a2av_in, a2av_out, rg, wvp, tp, ps)
                continue
            mw = 16 if m == 32 else 128
            wt = wp.tile([128, NK, 128], BF16, tag="wcat")
            nc.sync.dma_start(wt[:, :, :mw], wc3[:, :, ds(m * 128, mw)])
            pst = ps.tile([128, OT], FP32, tag="proj")
            for kt in range(NK):
                nc.tensor.matmul(pst[:mw, :], wt[:, kt, :mw], xhat[:, kt],
                                 start=(kt == 0), stop=(kt == NK - 1))
            if m < 8:
                nc.vector.tensor_scalar_mul(qT[:, m], pst[:], 1.0 / np.sqrt(dqk))
            elif m < 16:
                nc.vector.tensor_copy(kT[:, m - 8], pst[:])
            elif m < 32:
                nc.vector.tensor_copy(ogT[:, m - 16], pst[:])
            else:
                gt = tp.tile([16, OT], FP32, tag="gates")
                nc.scalar.activation(gt[:], pst[:16, :], AF.Exp,
                                     bias=b16[:], scale=-2.0 / CAP)
                nc.vector.tensor_scalar_add(gt[:], gt[:], 1.0)
                nc.vector.reciprocal(gt[:], gt[:])
                nc.vector.tensor_scalar(gt[:], gt[:], 2.0 * CAP, -CAP,
                                        AL.mult, AL.add)
                for hh in range(H):
                    nc.sync.dma_start(ag_g_in[hh], gt[ds(2 * hh, 2), :])
                nc.gpsimd.collective_compute(
                    "AllToAll", AL.bypass, replica_groups=rg,
                    ins=[ag_g_in[:]], outs=[ag_g_out[:]])
            if m == 15:
                # q and k projections done -> payloads + early A2A
                for hh in range(H):
                    nc.sync.dma_start(
                        a2a1_in[hh, ds(PAY_Q, 65536)].rearrange(
                            "(p t) -> p t", p=128), qT[:, hh])
                    nc.sync.dma_start(
                        a2a1_in[hh, ds(PAY_K, 65536)].rearrange(
                            "(p t) -> p t", p=128), kT[:, hh])
                nc.gpsimd.collective_compute(
                    "AllToAll", AL.bypass, replica_groups=rg,
                    ins=[a2a1_in[:]], outs=[a2a1_out[:]])

        # og payload + collective
        for hh in range(H):
            nc.sync.dma_start(
                a2aog_in[hh].rearrange(
                    "(half p t) -> p half t", half=2, p=128),
                ogT[:, ds(2 * hh, 2)])
        nc.gpsimd.collective_compute(
            "AllToAll", AL.bypass, replica_groups=rg,
            ins=[a2aog_in[:]], outs=[a2aog_out[:]])


def _v_proj(nc, xhat, wv_d, a2av_in, a2av_out, rg, wvp, tp, ps):
    """v = xhat.T @ wv, natural [own-t, D]; per-head payload + A2A."""
    for nb in range(4):          # D output chunks of 512
        wvt = wvp.tile([128, NK, 512], BF16, tag="wv")
        nc.sync.dma_start(
            wvt[:],
            wv_d.rearrange("(kt p) f -> p kt f", p=128)[:, :, ds(nb * 512, 512)])
        for tt in range(4):      # own-token tiles of 128
            pst = ps.tile([128, 512], FP32, tag="vproj")
            for kt in range(NK):
                nc.tensor.matmul(pst[:], xhat[:, kt, ds(tt * 128, 128)],
                                 wvt[:, kt], start=(kt == 0), stop=(kt == NK - 1))
            vsb = tp.tile([128, 512], BF16, tag="vsb")
            nc.vector.tensor_copy(vsb[:], pst[:])
            # dest head hh owns cols [hh*256, hh*256+256) of global D
            for hh in (2 * nb, 2 * nb + 1):
                off = hh * 256 - nb * 512
                nc.sync.dma_start(
                    a2av_in[hh, ds(tt * 128 * 256, 32768)]
                    .rearrange("(p c) -> p c", p=128),
                    vsb[:, ds(off, 256)])
    nc.gpsimd.collective_compute(
        "AllToAll", AL.bypass, replica_groups=rg,
        ins=[a2av_in[:]], outs=[a2av_out[:]])


def _attention(nc, tc, hout, a2a1_out, a2av_out, a2aog_out, ag_g_out, strip,
               ident, ones1f, onesb, wout, wtp, wps, rs_in, rs_out, rg):
    """head-c mLSTM over full S for b in {0,1}; fills hout [p, kt, b*4+tb, 512]."""
    with tc.tile_pool(name="at_in", bufs=1) as ip, \
         tc.tile_pool(name="at_tmp", bufs=2) as tp, \
         tc.tile_pool(name="at_row", bufs=1) as rp, \
         tc.tile_pool(name="at_psq", bufs=2, space="PSUM") as psq, \
         tc.tile_pool(name="at_psA", bufs=1, space="PSUM") as psA, \
         tc.tile_pool(name="at_ps1", bufs=1, space="PSUM") as ps1:
        qT_all = ip.tile([128, R, OT], BF16)
        nc.sync.dma_start(
            qT_all[:],
            a2a1_out[:, ds(PAY_Q, 65536)].rearrange("r (p t) -> p r t", p=128))
        kT_all = ip.tile([128, R, OT], BF16)
        nc.sync.dma_start(
            kT_all[:],
            a2a1_out[:, ds(PAY_K, 65536)].rearrange("r (p t) -> p r t", p=128))
        og_all = ip.tile([128, 2, R, OT], BF16)
        for half in range(2):
            nc.scalar.dma_start(
                og_all[:, half],
                a2aog_out[:, ds(half * 65536, 65536)].rearrange(
                    "r (p t) -> p r t", p=128))
        v_all = ip.tile([128, R, 4, 256], BF16)
        for tt in range(4):
            nc.scalar.dma_start(
                v_all[:, :, tt],
                a2av_out[:, ds(tt * 32768, 32768)].rearrange(
                    "r (p c) -> p r c", p=128))

        # gate rows as [1, X] partition-0 segments (ACT/DVE partition-offset
        # rules forbid odd base partitions); per-batch Fcum in its own tile.
        # G0 segments (x S): 0 irow_b0 | 1 irow_b1 | 2 frow_b0 | 3 frow_b1
        #                    | 4 work_b0 | 5 work_b1
        G0 = ip.tile([1, 6 * S], FP32)
        Fcum = [ip.tile([1, S], FP32, tag=f"Fcum{b}", name=f"Fcum{b}")
                for b in range(2)]
        for src in range(R):
            nc.scalar.dma_start(G0[:, ds(0 * S + OB * src, OB)],
                                ag_g_out[src, 0:1, 0:OB])
            nc.scalar.dma_start(G0[:, ds(2 * S + OB * src, OB)],
                                ag_g_out[src, 1:2, 0:OB])
            nc.scalar.dma_start(G0[:, ds(1 * S + OB * (7 - src), OB)],
                                ag_g_out[src, 0:1, OB:OT])
            nc.scalar.dma_start(G0[:, ds(3 * S + OB * (7 - src), OB)],
                                ag_g_out[src, 1:2, OB:OT])
        dpad = ip.tile([128, S], FP32)
        nc.vector.memset(dpad[:], 0.0)
        for b in range(2):
            wk = G0[:, ds((4 + b) * S, S)]
            nc.scalar.activation(wk, G0[:, ds((2 + b) * S, S)], AF.Exp, scale=-1.0)
            nc.vector.tensor_scalar_add(wk, wk, 1.0)
            nc.scalar.activation(wk, wk, AF.Ln)
            nc.vector.tensor_scalar_mul(wk, wk, -1.0)
            nc.vector.tensor_tensor_scan(Fcum[b][:], wk, wk, 0.0,
                                         AL.add, AL.bypass)
            # d = i - Fcum (reuse work segment), then DMA into dpad row b
            nc.vector.tensor_tensor(wk, G0[:, ds(b * S, S)], Fcum[b][:],
                                    AL.subtract)
            nc.scalar.dma_start(dpad[b:b + 1, :], wk)
        dcolT = ip.tile([128, 16, 2], FP32)
        for ck in range(16):
            tps = psq.tile([128, 128], FP32, tag="qk", name="tps")
            nc.tensor.transpose(tps[:], dpad[:, ds(ck * 128, 128)], ident[:])
            nc.vector.tensor_copy(dcolT[:, ck], tps[:, 0:2])

        for b in range(2):
            for tb in range(4):
                u = b * 4 + tb
                fb_ps = ps1.tile([128, 512], FP32, tag="r1", name="fb_ps")
                nc.tensor.matmul(fb_ps[:], ones1f[0:1, :],
                                 Fcum[b][:, ds(tb * 512, 512)],
                                 start=True, stop=True)
                A0 = psA.tile([128, 512], FP32, tag="A0")
                A1 = psA.tile([128, 512], FP32, tag="A1")
                n_ps = ps1.tile([1, 512], FP32, tag="r2", name="n_ps")
                nst = 4 * tb + 4
                for g in range(nst):
                    src = (g // 2) if b == 0 else (7 - g // 2)
                    co = (g % 2) * 128 + b * OB       # col offset in payload
                    qk = psq.tile([128, 512], FP32, tag="qk")
                    if b == 0:
                        nc.tensor.matmul(qk[:], kT_all[:, src, ds(co, 128)],
                                         qT_all[:, ds(2 * tb, 2), 0:OB],
                                         start=True, stop=True)
                    else:
                        nc.tensor.matmul(qk[:, 0:256], kT_all[:, src, ds(co, 128)],
                                         qT_all[:, 7 - 2 * tb, OB:OT],
                                         start=True, stop=True)
                        nc.tensor.matmul(qk[:, 256:512], kT_all[:, src, ds(co, 128)],
                                         qT_all[:, 6 - 2 * tb, OB:OT],
                                         start=True, stop=True)
                    sexp = tp.tile([128, 512], FP32, tag="sexp")
                    nc.scalar.activation(sexp[:], fb_ps[:], AF.Exp,
                                         bias=dcolT[:, g, b:b + 1])
                    cp = tp.tile([128, 512], BF16, tag="cp")
                    nc.vector.tensor_tensor(cp[:], qk[:], sexp[:], AL.mult)
                    if g >= 4 * tb:
                        kk = g - 4 * tb
                        nc.vector.tensor_tensor(
                            cp[:], cp[:], strip[:, ds((3 - kk) * 128, 512)], AL.mult)
                    nc.tensor.matmul(n_ps[:], onesb[:], cp[:],
                                     start=(g == 0), stop=(g == nst - 1))
                    vi = 2 * b + (g % 2)
                    nc.tensor.matmul(A0[:], v_all[:, src, vi, ds(0, 128)], cp[:],
                                     start=(g == 0), stop=(g == nst - 1))
                    nc.tensor.matmul(A1[:], v_all[:, src, vi, ds(128, 128)], cp[:],
                                     start=(g == 0), stop=(g == nst - 1))
                # free the A psum banks early: stage to bf16 SBUF
                Acp = [rp.tile([128, 512], BF16, tag=f"Acp{h2}", name=f"Acp{h2}")
                       for h2 in range(2)]
                nc.vector.tensor_copy(Acp[0][:], A0[:])
                nc.vector.tensor_copy(Acp[1][:], A1[:])
                # n = max(|sum C|, 1);  srow = rsqrt(ssqA/dv + EPS*n^2)
                nn = rp.tile([1, 512], FP32, tag="nn")
                nc.vector.tensor_scalar_mul(nn[:], n_ps[:], -1.0)
                nc.vector.tensor_tensor(nn[:], nn[:], n_ps[:], AL.max)
                nc.vector.tensor_scalar_max(nn[:], nn[:], 1.0)
                ssq_ps = ps1.tile([1, 512], FP32, tag="r2", name="ssqA_ps")
                for half in range(2):
                    asq = tp.tile([128, 512], BF16, tag="asq")
                    nc.scalar.activation(asq[:], Acp[half][:], AF.Square)
                    nc.tensor.matmul(ssq_ps[:], onesb[:], asq[:],
                                     start=(half == 0), stop=(half == 1))
                srow = rp.tile([1, 512], FP32, tag="srow")
                nc.vector.tensor_tensor(srow[:], nn[:], nn[:], AL.mult)
                nc.vector.tensor_scalar_mul(srow[:], srow[:], EPS)
                nc.vector.tensor_scalar_mul(nn[:], ssq_ps[:], 1.0 / dv)
                nc.vector.tensor_tensor(srow[:], srow[:], nn[:], AL.add)
                nc.scalar.activation(srow[:], srow[:], AF.Ln)
                nc.scalar.activation(srow[:], srow[:], AF.Exp, scale=-0.5)
                sb_ps = rp.tile([128, 512], FP32, tag="sbb", name="sb_sb")
                nc.gpsimd.partition_broadcast(sb_ps[:], srow[:])
                for half in range(2):
                    sig = tp.tile([128, 512], FP32, tag="sig")
                    if b == 0:
                        nc.scalar.activation(sig[:],
                                             og_all[:, half, ds(2 * tb, 2), 0:OB],
                                             AF.Exp, scale=-1.0)
                    else:
                        nc.scalar.activation(sig[:, 0:256],
                                             og_all[:, half, 7 - 2 * tb, OB:OT],
                                             AF.Exp, scale=-1.0)
                        nc.scalar.activation(sig[:, 256:512],
                                             og_all[:, half, 6 - 2 * tb, OB:OT],
                                             AF.Exp, scale=-1.0)
                    nc.vector.tensor_scalar_add(sig[:], sig[:], 1.0)
                    nc.vector.reciprocal(sig[:], sig[:])
                    tmp = tp.tile([128, 512], FP32, tag="hmul")
                    nc.vector.tensor_tensor(tmp[:], Acp[half][:], sig[:], AL.mult)
                    nc.vector.tensor_tensor(hout[:, half, u], tmp[:], sb_ps[:],
                                            AL.mult)
            _wout_phase(nc, tc, wout, wtp, wps, hout, rs_in[b], b)
            nc.gpsimd.collective_compute(
                "ReduceScatter", AL.add, replica_groups=rg,
                ins=[rs_in[b][:]], outs=[rs_out[b][:]])


def _wout_phase(nc, tc, wout, tp, ps, hout, rs_in_b, b):
    """mix partial = w_out[head rows].T @ hout for batch b; scatter to rs_in."""
    for tb in range(4):
        u = b * 4 + tb
        for m in range(NK):
            pst = ps.tile([128, 512], FP32, tag="wo")
            for kt in range(2):
                nc.tensor.matmul(pst[:], wout[:, kt, ds(m * 128, 128)],
                                 hout[:, kt, u],
                                 start=(kt == 0), stop=(kt == 1))
            st = tp.tile([128, 512], BF16, tag="wostage")
            nc.vector.tensor_copy(st[:], pst[:])
            for hf in range(2):
                gblk = 2 * tb + hf
                dest = gblk if b == 0 else 7 - gblk
                nc.scalar.dma_start(
                    rs_in_b[dest, ds(m * 128, 128), :],
                    st[:, ds(hf * 256, 256)])


def _x1_phase(nc, tc, xT, rs_out, ox1_d, ag2_in, ones1f, onesb):
    with tc.tile_pool(name="x1_p", bufs=1) as pp, \
         tc.tile_pool(name="x1_tmp", bufs=3) as tp, \
         tc.tile_pool(name="x1_ps", bufs=1, space="PSUM") as ps:
        x1T = pp.tile([128, NK, OT], FP32)
        h2T = pp.tile([128, NK, OT], BF16)
        for b in range(2):
            tc_ = ds(b * OB, OB)
            rsb = tp.tile([128, NK, OB], BF16, tag="rsb")
            nc.scalar.dma_start(rsb[:],
                                rs_out[b].rearrange("(kt p) t -> p kt t", p=128))
            nc.vector.tensor_tensor(x1T[:, :, tc_], xT[:, :, tc_], rsb[:], AL.add)
            nc.sync.dma_start(
                ox1_d.rearrange("(kt p) t -> p kt t", p=128)[:, :, tc_],
                x1T[:, :, tc_])
            ssq_ps = ps.tile([1, OB], FP32, tag="ssq2")
            for kt in range(NK):
                sq = tp.tile([128, OB], BF16, tag="sq2")
                nc.scalar.activation(sq[:], x1T[:, kt, tc_], AF.Square)
                nc.tensor.matmul(ssq_ps[:], onesb[:], sq[:],
                                 start=(kt == 0), stop=(kt == NK - 1))
            scl = tp.tile([1, OB], FP32, tag="scl2")
            nc.vector.tensor_scalar(scl[:], ssq_ps[:], 1.0 / D, EPS,
                                    AL.mult, AL.add)
            nc.scalar.activation(scl[:], scl[:], AF.Ln)
            nc.scalar.activation(scl[:], scl[:], AF.Exp, scale=-0.5)
            sc_ps = ps.tile([128, OB], FP32, tag="sc2")
            nc.tensor.matmul(sc_ps[:], ones1f[0:1, :], scl[:],
                             start=True, stop=True)
            for kt in range(NK):
                nc.vector.tensor_tensor(h2T[:, kt, tc_], x1T[:, kt, tc_],
                                        sc_ps[:], AL.mult)
            nc.scalar.dma_start(
                ag2_in.rearrange("(kt p) t -> p kt t", p=128)[:, :, tc_],
                h2T[:, :, tc_])


def _mlp_phase(nc, tc, wg_d, wu_d, wd_d, ag2_out, omlp_d):
    with tc.tile_pool(name="ml_w", bufs=1) as wp, \
         tc.tile_pool(name="ml_h", bufs=2) as hp, \
         tc.tile_pool(name="ml_tmp", bufs=3) as tp, \
         tc.tile_pool(name="ml_ps", bufs=1, space="PSUM") as ps, \
         tc.tile_pool(name="ml_psgu", bufs=2, space="PSUM") as psgu:
        wg = wp.tile([128, NK, FFC], BF16)
        nc.sync.dma_start(wg[:], wg_d.rearrange("(kt p) f -> p kt f", p=128))
        wu = wp.tile([128, NK, FFC], BF16)
        nc.sync.dma_start(wu[:], wu_d.rearrange("(kt p) f -> p kt f", p=128))
        wd = wp.tile([128, 8, D], BF16)
        nc.sync.dma_start(wd[:], wd_d.rearrange("(kt p) f -> p kt f", p=128))
        for rb in range(R):
            h2b = hp.tile([128, NK, OT], BF16, tag="h2b")
            nc.scalar.dma_start(h2b[:],
                                ag2_out[rb].rearrange("(kt p) t -> p kt t", p=128))
            ga = hp.tile([128, 8, OT], BF16, tag="ga")
            aa = hp.tile([128, 8, OT], BF16, tag="aa")
            for mf in range(8):
                gps = psgu.tile([128, OT], FP32, tag="g")
                for kt in range(NK):
                    nc.tensor.matmul(gps[:], wg[:, kt, ds(mf * 128, 128)],
                                     h2b[:, kt],
                                     start=(kt == 0), stop=(kt == NK - 1))
                nc.scalar.activation(ga[:, mf], gps[:], AF.Silu)
                ups = psgu.tile([128, OT], FP32, tag="u")
                for kt in range(NK):
                    nc.tensor.matmul(ups[:], wu[:, kt, ds(mf * 128, 128)],
                                     h2b[:, kt],
                                     start=(kt == 0), stop=(kt == NK - 1))
                nc.vector.tensor_tensor(aa[:, mf], ups[:], ga[:, mf], AL.mult)
            for tt in range(4):
                opss = [ps.tile([128, 512], FP32, tag=f"o{nb}", name=f"o{nb}")
                        for nb in range(4)]
                for kt in range(8):
                    for nb in range(4):
                        nc.tensor.matmul(opss[nb][:], aa[:, kt, ds(tt * 128, 128)],
                                         wd[:, kt, ds(nb * 512, 512)],
                                         start=(kt == 0), stop=(kt == 7),
                                         skip_group_check=True)
                for nb in range(4):
                    ost = tp.tile([128, 512], BF16, tag="ost")
                    nc.vector.tensor_copy(ost[:], opss[nb][:])
                    nc.sync.dma_start(
                        omlp_d[ds(rb * OT + tt * 128, 128), ds(nb * 512, 512)],
                        ost[:])


def _build():
    nc = bacc.Bacc(num_devices=R)
    rg = [list(range(R))]

    xT_d = nc.dram_tensor("xT", [D, OT], FP32, kind="ExternalInput")
    wcat_d = nc.dram_tensor("wcat", [D, WCOLS], BF16, kind="ExternalInput")
    wv_d = nc.dram_tensor("wv", [D, D], BF16, kind="ExternalInput")
    b16_d = nc.dram_tensor("b16", [16, 1], FP32, kind="ExternalInput")
    wout_d = nc.dram_tensor("wout", [dv, D], BF16, kind="ExternalInput")
    wg_d = nc.dram_tensor("wg", [D, FFC], BF16, kind="ExternalInput")
    wu_d = nc.dram_tensor("wu", [D, FFC], BF16, kind="ExternalInput")
    wd_d = nc.dram_tensor("wd", [FFC, D], BF16, kind="ExternalInput")
    strip_d = nc.dram_tensor("strip", [128, 896], BF16, kind="ExternalInput")
    ident_d = nc.dram_tensor("ident", [128, 128], FP32, kind="ExternalInput")
    ones1f_d = nc.dram_tensor("ones1f", [65, 128], FP32, kind="ExternalInput")
    onesb_d = nc.dram_tensor("onesb", [128, 1], BF16, kind="ExternalInput")

    ox1_d = nc.dram_tensor("out_x1", [D, OT], FP32, kind="ExternalOutput")
    omlp_d = nc.dram_tensor("out_mlp", [R * OT, D], BF16, kind="ExternalOutput")

    a2a1_in = nc.dram_tensor("a2a1_in", [R, PAY1], BF16)
    a2a1_out = nc.dram_tensor("a2a1_out", [R, PAY1], BF16)
    a2av_in = nc.dram_tensor("a2av_in", [R, PAYV], BF16)
    a2av_out = nc.dram_tensor("a2av_out", [R, PAYV], BF16)
    a2aog_in = nc.dram_tensor("a2aog_in", [R, PAYOG], BF16)
    a2aog_out = nc.dram_tensor("a2aog_out", [R, PAYOG], BF16)
    ag_g_in = nc.dram_tensor("ag_g_in", [R, 2, OT], FP32)
    ag_g_out = nc.dram_tensor("ag_g_out", [R, 2, OT], FP32)
    rs_in = [nc.dram_tensor(f"rs_in{b}", [R, D, OB], BF16) for b in range(2)]
    rs_out = [nc.dram_tensor(f"rs_out{b}", [D, OB], BF16) for b in range(2)]
    ag2_in = nc.dram_tensor("ag2_in", [D, OT], BF16)
    ag2_out = nc.dram_tensor("ag2_out", [R, D, OT], BF16, addr_space="Shared")

    with TileContext(nc) as tc:
        with tc.tile_pool(name="glob", bufs=1) as gp:
            strip = gp.tile([128, 896], BF16)
            nc.sync.dma_start(strip[:], strip_d[:])
            ident = gp.tile([128, 128], FP32)
            nc.sync.dma_start(ident[:], ident_d[:])
            ones1f = gp.tile([65, 128], FP32)
            nc.sync.dma_start(ones1f[:], ones1f_d[:])
            onesb = gp.tile([128, 1], BF16)
            nc.sync.dma_start(onesb[:], onesb_d[:])
            b16 = gp.tile([16, 1], FP32)
            nc.sync.dma_start(b16[:], b16_d[:])

            with tc.tile_pool(name="mid", bufs=1) as mp:
                xT = mp.tile([128, NK, OT], FP32)
                nc.sync.dma_start(xT[:],
                                  xT_d.rearrange("(kt p) t -> p kt t", p=128))
                hout = mp.tile([128, 2, 8, OT], BF16)  # [p, dvhalf, b*4+tb, t]

                _mixer(nc, tc, xT, wcat_d, wv_d, b16, a2a1_in, a2a1_out,
                       a2av_in, a2av_out, a2aog_in, a2aog_out,
                       ag_g_in, ag_g_out, rg, onesb, ones1f)

                with tc.tile_pool(name="wo_w", bufs=1) as wp, \
                     tc.tile_pool(name="wo_tmp", bufs=3) as wtp, \
                     tc.tile_pool(name="wo_ps", bufs=2, space="PSUM") as wps:
                    wout = wp.tile([128, 2, D], BF16)
                    nc.sync.dma_start(
                        wout[:], wout_d.rearrange("(kt p) f -> p kt f", p=128))
                    _attention(nc, tc, hout, a2a1_out, a2av_out, a2aog_out,
                               ag_g_out, strip, ident, ones1f, onesb,
                               wout, wtp, wps, rs_in, rs_out, rg)
                _x1_phase(nc, tc, xT, rs_out, ox1_d, ag2_in, ones1f, onesb)
            nc.gpsimd.collective_compute(
                "AllGather", AL.bypass, replica_groups=rg,
                ins=[ag2_in[:]], outs=[ag2_out[:]])
            _mlp_phase(nc, tc, wg_d, wu_d, wd_d, ag2_out, omlp_d)

    nc.finalize()
    return nc


_NC_CACHE = None


def kernel(x, norm1_w, wq, wk, wv, w_ig, b_ig, w_fg, b_fg, w_og, mh_w,
           w_out, norm2_w, w_gate, w_up, w_down):
    global _NC_CACHE
    x = np.asarray(x, np.float32)
    n1 = np.asarray(norm1_w, np.float32)
    n2 = np.asarray(norm2_w, np.float32)
    mh = np.asarray(mh_w, np.float32)

    wif = np.empty((D, 2 * H), np.float32)
    wif[:, 0::2] = np.asarray(w_ig)
    wif[:, 1::2] = np.asarray(w_fg)
    b16v = np.empty((16, 1), np.float32)
    b16v[0::2, 0] = -2.0 * np.asarray(b_ig) / CAP
    b16v[1::2, 0] = -2.0 * np.asarray(b_fg) / CAP

    wcat = (np.concatenate([np.asarray(wq), np.asarray(wk), np.asarray(w_og), wif],
                           axis=1) * n1[:, None]).astype(bf16)
    wv_b = (np.asarray(wv) * n1[:, None]).astype(bf16)
    wout_f = np.asarray(w_out) * mh[:, None]
    wg_f = (np.asarray(w_gate) * n2[:, None]).astype(bf16)
    wu_f = (np.asarray(w_up) * n2[:, None]).astype(bf16)
    wd_b = np.asarray(w_down).astype(bf16)

    i_idx = np.arange(128)[:, None]
    c_idx = np.arange(896)[None, :]
    strip = ((c_idx - i_idx) >= 384).astype(bf16)
    ident = np.eye(128, dtype=np.float32)
    ones1f = np.ones((65, 128), np.float32)
    onesb = np.ones((128, 1), bf16)

    in_maps = []
    for c in range(R):
        s0 = slice(OB * c, OB * (c + 1))
        s1 = slice(OB * (7 - c), OB * (8 - c))
        xT = np.ascontiguousarray(
            np.concatenate([x[0, s0].T, x[1, s1].T], axis=1)).astype(np.float32)
        in_maps.append({
            "xT": xT, "wcat": wcat, "wv": wv_b, "b16": b16v,
            "wout": np.ascontiguousarray(wout_f[dv * c:dv * (c + 1)]).astype(bf16),
            "wg": np.ascontiguousarray(wg_f[:, FFC * c:FFC * (c + 1)]),
            "wu": np.ascontiguousarray(wu_f[:, FFC * c:FFC * (c + 1)]),
            "wd": np.ascontiguousarray(wd_b[FFC * c:FFC * (c + 1)]),
            "strip": strip, "ident": ident, "ones1f": ones1f, "onesb": onesb,
        })

    if _NC_CACHE is None:
        _NC_CACHE = _build()
    res = run_bass_kernel_spmd(_NC_CACHE, in_maps, core_ids=list(range(R)))

    out = np.zeros((B, S, D), np.float32)
    for c in range(R):
        x1T = np.asarray(res.results[c]["out_x1"]).astype(np.float32)
        s0 = slice(OB * c, OB * (c + 1))
        s1 = slice(OB * (7 - c), OB * (8 - c))
        out[0, s0] = x1T[:, :OB].T
        out[1, s1] = x1T[:, OB:].T
    mlp = np.zeros((R * OT, D), np.float32)
    for c in range(R):
        mlp += np.asarray(res.results[c]["out_mlp"]).astype(np.float32)
    for r in range(R):
        blk = mlp[r * OT:(r + 1) * OT]
        out[0, OB * r:OB * (r + 1)] += blk[:OB]
        out[1, OB * (7 - r):OB * (8 - r)] += blk[OB:]
    return out

